# revision 3
# baseline (speedup 1.0000x reference)
"""Trainium2 Bass kernel for nn_DenoisingModule (non-local attention block).

Reference computation (per batch element n, with C=256 channels, HW=4096):
    theta = W_t x + b_t            # queries  [C, HW]
    phi   = W_p x + b_p            # keys     [C, HW]
    g     = x                      # values   [C, HW]
    S     = theta^T phi / sqrt(C)  # [HW, HW]
    A     = softmax(S, axis=keys)
    f     = g A^T                  # [C, HW]
    out   = x + W_c f + b_c

Sharding: 8 cores; each of the N=4 batch elements is split across 2 cores
by query position (2048 queries per core). Every core holds the full key
set for its batch element, so no collectives are needed.

Per-core device program (SPMD, identical on all cores, data differs):
  - ALL matmuls run in fp8e4 DoubleRow mode (2 contraction k-tiles per
    instruction, 2 fp8 MACs/cell/cycle): the 256-channel contractions
    (projections, scores) are one DR matmul each; the 4096-key PV
    contraction is 16 DR matmuls per 512-query group.
  - scores are computed TRANSPOSED (S^T[k, q] = phi^T theta) so the exp
    output E^T feeds the PV matmul directly as the fp8 moving operand.
  - softmax uses exp(s - 3) (constant shift, softmax-invariant) so the
    max exp value ~96 stays under the TRN fp8e4 max of 240, and typical
    weights ~0.08 stay in the normal range.
  - the softmax denominator is a ones-column DR matmul per exp pair,
    accumulated in PSUM across the key loop (frees the Vector engine,
    which was the previous denominator path).
  - scale management for fp8: host prescales W by 16 (keeps W out of the
    fp8 subnormal range); evictions divide by 16 on DVE. f is scaled by
    16/sum (reciprocal folded with ln16 bias on ACT); the conv PSUM is
    divided by 256 in the final residual-add STT.
  - theta/phi evictions run on the Vector engine (tensor_scalar with
    1/16 scale + bias); exp and the reciprocal ln/exp run on ACT.
  - the attention loop is software-pipelined (PV trails scores/exp by one
    pair; per-group normalize/conv work is deferred into the next group).

Toolchain constraint that shapes this file: every TPB engine instruction
(and every DMA) may carry at most ONE semaphore wait, so cross-engine
fan-in is funneled through per-engine collector chains, persistent ring
tiles replace rotating tile pools, and loads/stores are merged so each
DMA is the first instruction on its hardware queue.

The host wrapper rolls x columns per-core so queries are always columns
[0, P) of the local key matrix (keeps the program identical across cores),
and pre-transposes x (and the weight matrices) since the PV matmul needs
x^T as the stationary operand.
"""

import math

import numpy as np

import concourse.bass as bass
import concourse.mybir as mybir
from concourse import bacc
from concourse.bass_utils import run_bass_kernel_spmd
from concourse.tile import TileContext, add_dep_helper

N, C, H, W = 4, 256, 64, 64
HW = H * W
NCORES = 8
CORES_PER_N = NCORES // N
P_CORE = HW // CORES_PER_N  # queries per core

F32 = mybir.dt.float32
F16 = mybir.dt.float16
FP8 = mybir.dt.float8e4
DR = mybir.MatmulPerfMode.DoubleRow

EXP_SHIFT = -3.0  # softmax-invariant shift keeping exp under fp8e4 max 240
W_SCALE = 16.0    # host premultiplies weights (fp8 subnormal avoidance)
F_SCALE = 16.0    # f is stored as 16*f_normalized


def build_program(P, Q, Cc=C):
    """Build the per-core Bass program.

    P: queries handled by this core (first P columns of xk)
    Q: total key positions
    """
    assert P % 512 == 0 and Q % 512 == 0 and Cc == 256
    CT = Cc // 128  # = 2: one DoubleRow pair
    QT = Q // 128
    PG = P // 512
    QG = Q // 512
    scale = float(Cc) ** -0.5

    nc = bacc.Bacc("TRN2", target_bir_lowering=False)
    xk = nc.declare_dram_parameter("xk", [Cc, Q], FP8, isOutput=False)[:]
    xqb = nc.declare_dram_parameter("xqb", [Cc, P], F16, isOutput=False)[:]
    xt = nc.declare_dram_parameter("xt", [Q, Cc], FP8, isOutput=False)[:]
    wcat = nc.declare_dram_parameter("wcat", [3, Cc, Cc], FP8, isOutput=False)[:]
    bcat = nc.declare_dram_parameter("bcat", [3, Cc], F32, isOutput=False)[:]
    out = nc.declare_dram_parameter("out", [Cc, P], F16, isOutput=True)[:]

    add = mybir.AluOpType.add
    mult = mybir.AluOpType.mult

    with TileContext(nc) as tc:
        with (
            tc.tile_pool(name="const", bufs=1) as const,
            tc.tile_pool(name="big", bufs=1) as big,
            tc.tile_pool(name="pss", bufs=1, space="PSUM") as pss,
            tc.tile_pool(name="psf", bufs=1, space="PSUM") as psf,
            tc.tile_pool(name="pso", bufs=1, space="PSUM") as pso,
        ):
            QH = Q // 2

            # ---- input loads: 6 DMAs, one per HWDGE proc ----
            ws_sb = const.tile([128, 3, CT, Cc], FP8, tag="ws")
            w_load = nc.sync.dma_start(
                out=ws_sb, in_=wcat.rearrange("w (a p) o -> p w a o", p=128)
            )
            xk_sb = big.tile([128, CT, Q], FP8, tag="xk")
            xk_loads = [
                nc.sync.dma_start(
                    out=xk_sb[:, :, h * QH : (h + 1) * QH],
                    in_=xk[:, h * QH : (h + 1) * QH].rearrange(
                        "(a p) q -> p a q", p=128
                    ),
                )
                for h in range(2)
            ]
            bb = const.tile([128, 3, CT], F32, tag="bb")
            b_load = nc.sync.dma_start(
                out=bb, in_=bcat.rearrange("w (a p) -> p w a", p=128)
            )
            xq_sb = big.tile([128, CT, P], F16, tag="xq")
            xq_load = nc.sync.dma_start(
                out=xq_sb, in_=xqb.rearrange("(a p) q -> p a q", p=128)
            )
            xt_sb = big.tile([128, QT, Cc], FP8, tag="xt")
            xt_load = nc.sync.dma_start(
                out=xt_sb, in_=xt.rearrange("(a p) c -> p a c", p=128)
            )

            # persistent tiles (deliberately NOT pool-rotated: pool-slot
            # releases fan in multiple procs; rings keep wait fan-in low)
            th_sb = big.tile([128, CT, P], FP8, tag="th")
            ph_sb = big.tile([128, CT, Q], FP8, tag="ph")
            f_sb = big.tile([128, CT, P], FP8, tag="f")
            e_ring = big.tile([128, 4, 2, 512], FP8, tag="ering")
            rc_ring = const.tile([1, PG, 512], F16, tag="rcring")
            lns = const.tile([1, PG, 512], F32, tag="lns")
            bc_ring = big.tile([128, PG, 512], F32, tag="bcring")
            o_ring = big.tile([128, CT, PG, 512], F16, tag="oring")

            # ---- engine program-order chains + wait collectors ----
            last = {}

            def chain(eng, inst):
                # ordering edges disabled: Bacc legalizes multi-waits, so the
                # Tile scheduler is free to interleave within each engine
                last[eng] = inst.ins
                return inst

            ones_f = const.tile([128, 1], F32, tag="ones_f")
            chain("v", nc.vector.memset(ones_f, 1.0))
            ones_col = const.tile([1, 128], F16, tag="ones_col")
            chain("v", nc.vector.tensor_copy(
                ones_col, ones_f[0:1, 0:1].to_broadcast([1, 128])))
            ones8 = const.tile([128, 2, 16], FP8, tag="ones8")
            chain("v", nc.vector.memset(ones8, 1.0))
            zbias = const.tile([128, 1], F32, tag="zbias")
            zb_inst = chain("v", nc.vector.memset(zbias, 0.0))
            mbias = const.tile([128, 1], F32, tag="mbias")
            chain("v", nc.vector.memset(mbias, EXP_SHIFT))
            lbias = const.tile([128, 1], F32, tag="lbias")
            chain("v", nc.vector.memset(lbias, math.log(F_SCALE)))

            scr_act = const.tile([1, 1], F32, tag="scr_act")
            acol = nc.scalar.activation(
                scr_act, zbias[0:1, :], mybir.ActivationFunctionType.Copy
            )
            add_dep_helper(acol.ins, zb_inst.ins, True, "act bias barrier")
            last["a"] = acol.ins

            for k, ld in enumerate([b_load, xq_load]):
                scr_k = const.tile([1, 1], F32, tag=f"scr{k}", name=f"scr{k}")
                dcol = nc.vector.memset(scr_k, 0.0)
                add_dep_helper(dcol.ins, ld.ins, True, "dve input barrier")
                chain("v", dcol)

            ps_col = pso.tile([1, 1], F32, tag="den", name="ps_col")
            probe = bb[0:1, 0, 0:1]

            def pe_barrier(ld):
                col = nc.tensor.matmul(ps_col, lhsT=probe, rhs=probe)
                add_dep_helper(col.ins, ld.ins, True, "pe input barrier")
                chain("p", col)

            pe_barrier(w_load)
            pe_barrier(xk_loads[0])

            def mm(*args, **kwargs):
                return chain("p", nc.tensor.matmul(*args, **kwargs))

            def dve(fn, *args, **kwargs):
                return chain("v", fn(*args, **kwargs))

            def act(*args, **kwargs):
                return chain("a", nc.scalar.activation(*args, **kwargs))

            # ---- projections: one DR matmul per 512-col group; DVE
            # eviction applies the 1/16 weight-prescale compensation + bias
            def project(w_idx, dst, ngroups, bias_col, g0=0):
                for co in range(CT):
                    for g in range(g0, ngroups):
                        ps_pj = psf.tile(
                            [128, 512], F32, tag=f"f{g % 2}", name="ps_pj"
                        )
                        mm(
                            ps_pj,
                            lhsT=ws_sb[:, w_idx, :, co * 128 : (co + 1) * 128],
                            rhs=xk_sb[:, :, g * 512 : (g + 1) * 512],
                            perf_mode=DR,
                        )
                        dve(
                            nc.vector.tensor_scalar,
                            dst[:, co, g * 512 : (g + 1) * 512],
                            ps_pj,
                            1.0 / W_SCALE,
                            bb[:, bias_col, co : co + 1],
                            op0=mult,
                            op1=add,
                        )

            project(0, th_sb, PG, 0)
            project(1, ph_sb, QG // 2, 1)
            pe_barrier(xk_loads[1])
            project(1, ph_sb, QG, 1, g0=QG // 2)
            pe_barrier(xt_load)

            # ---- attention; per-group finalization is deferred into the
            # next group so the reciprocal chain (PSUM den -> ACT ln/exp ->
            # PE broadcast) overlaps PE work
            deferred = [None]

            def finalize_bc(pg):
                psl = slice(pg * 512, (pg + 1) * 512)
                ps_fs = deferred[0][1]
                ps_bc = pso.tile([128, 512], F32, tag="po", name="ps_bc")
                mm(ps_bc, lhsT=ones_col, rhs=rc_ring[:, pg, :])
                bc_sb = bc_ring[:, pg, :]
                dve(nc.vector.tensor_copy, bc_sb, ps_bc)
                for ci in range(CT):
                    dve(
                        nc.vector.tensor_mul, f_sb[:, ci, psl], ps_fs[ci], bc_sb
                    )

            def finalize_conv(pg):
                psl = slice(pg * 512, (pg + 1) * 512)
                for co in range(CT):
                    ps_o = pso.tile([128, 512], F32, tag="po", name="ps_o")
                    mm(
                        ps_o,
                        lhsT=ws_sb[:, 2, :, co * 128 : (co + 1) * 128],
                        rhs=f_sb[:, :, psl],
                        perf_mode=DR,
                    )
                    dve(
                        nc.vector.scalar_tensor_tensor,
                        out=o_ring[:, co, pg, :],
                        in0=ps_o,
                        scalar=1.0 / (W_SCALE * F_SCALE),
                        in1=xq_sb[:, co, psl],
                        op0=mult,
                        op1=add,
                    )
                deferred[0] = None

            for pg in range(PG):
                psl = slice(pg * 512, (pg + 1) * 512)
                ps_f = [
                    psf.tile([128, 512], F32, tag=f"f{ci}", name=f"ps_f{ci}")
                    for ci in range(CT)
                ]
                ps_den = pso.tile([1, 512], F32, tag="den", name="ps_den")
                # software pipeline: PV runs one exp-pair behind scores so
                # the PE streams scores(k+1) while ACT computes exp(k)
                for qp in range(QT // 2 + 1):
                    if qp < QT // 2:
                        ps_s = pss.tile([128, 2, 512], F32, tag="s", bufs=2)
                        for sub in range(2):
                            qt = qp * 2 + sub
                            mm(
                                ps_s[:, sub],
                                lhsT=ph_sb[:, :, qt * 128 : (qt + 1) * 128],
                                rhs=th_sb[:, :, psl],
                                perf_mode=DR,
                            )
                        act(
                            e_ring[:, qp % 4], ps_s,
                            mybir.ActivationFunctionType.Exp,
                            bias=mbias, scale=scale,
                        )
                    if qp == 1 and deferred[0] is not None:
                        finalize_bc(pg - 1)
                    if qp == 2 and deferred[0] is not None:
                        finalize_conv(pg - 1)
                    if qp >= 1:
                        qpp = qp - 1
                        e_p = e_ring[:, qpp % 4]
                        first, last_q = qpp == 0, qpp == QT // 2 - 1
                        mm(
                            ps_den,
                            lhsT=ones8[:, :, 0:1],
                            rhs=e_p,
                            perf_mode=DR,
                            start=first,
                            stop=last_q,
                        )
                        for ci in range(CT):
                            mm(
                                ps_f[ci],
                                lhsT=xt_sb[
                                    :, 2 * qpp : 2 * qpp + 2,
                                    ci * 128 : (ci + 1) * 128,
                                ],
                                rhs=e_p,
                                perf_mode=DR,
                                start=first,
                                stop=last_q,
                            )

                # reciprocal: 16/sum via ln+exp (one ACT table set)
                act(
                    lns[:, pg], ps_den, mybir.ActivationFunctionType.Ln,
                    bias=zbias[0:1],
                )
                act(
                    rc_ring[:, pg, :], lns[:, pg],
                    mybir.ActivationFunctionType.Exp,
                    bias=lbias[0:1], scale=-1.0,
                )
                deferred[0] = (pg, ps_f)

            finalize_bc(PG - 1)
            finalize_conv(PG - 1)

            # ---- output stores: one contiguous DMA per channel tile ----
            for co in range(CT):
                nc.sync.dma_start(
                    out=out[co * 128 : (co + 1) * 128, :], in_=o_ring[:, co]
                )
    nc.compile()
    return nc


_PROGRAM_CACHE = {}


def _get_program(mm_dt=None):
    key = "fp8dr"
    if key not in _PROGRAM_CACHE:
        _PROGRAM_CACHE[key] = build_program(P_CORE, HW, C)
    return _PROGRAM_CACHE[key]


def make_in_maps(x, theta_w, theta_b, phi_w, phi_b, conv1_w, conv1_b,
                 mm_np=None):
    """Host-side sharding / layout prep (pure data movement + prescale)."""
    fp8 = mybir.dt.np(FP8)
    wcat = np.ascontiguousarray(
        np.clip(
            W_SCALE * np.stack(
                [
                    np.asarray(theta_w, np.float32).T,
                    np.asarray(phi_w, np.float32).T,
                    np.asarray(conv1_w, np.float32).T,
                ]
            ),
            -240.0, 240.0,
        ).astype(fp8)
    )
    bcat = np.ascontiguousarray(
        np.stack(
            [
                np.asarray(theta_b, np.float32),
                np.asarray(phi_b, np.float32),
                np.asarray(conv1_b, np.float32),
            ]
        )
    )
    xf = np.asarray(x, np.float32).reshape(N, C, HW)
    cb = np.asarray(conv1_b, np.float32)[:, None]
    in_maps = []
    for core in range(NCORES):
        n, half = divmod(core, CORES_PER_N)
        off = half * P_CORE
        xk_i = np.ascontiguousarray(np.roll(xf[n], -off, axis=1))
        in_maps.append(
            {
                "xk": xk_i.astype(fp8),
                "xqb": (xk_i[:, :P_CORE] + cb).astype(np.float16),
                "xt": np.ascontiguousarray(xk_i.T).astype(fp8),
                "wcat": wcat,
                "bcat": bcat,
            }
        )
    return in_maps


def assemble_output(results):
    y = np.empty((N, C, HW), np.float32)
    for core in range(NCORES):
        n, half = divmod(core, CORES_PER_N)
        off = half * P_CORE
        y[n][:, off : off + P_CORE] = results[core]["out"].astype(np.float32)
    return y.reshape(N, C, H, W)


def kernel(x, theta_w, theta_b, phi_w, phi_b, conv1_w, conv1_b,
           mm_dt=None, **run_kwargs):
    nc = _get_program()
    in_maps = make_in_maps(
        x, theta_w, theta_b, phi_w, phi_b, conv1_w, conv1_b
    )
    res = run_bass_kernel_spmd(nc, in_maps, list(range(NCORES)), **run_kwargs)
    out = assemble_output(res.results)
    kernel.last_results = res
    return out


# revision 6
# speedup vs baseline: 1.1438x; 1.1438x over previous
"""Trainium2 Bass kernel for nn_DenoisingModule (non-local attention block).

Reference computation (per batch element n, with C=256 channels, HW=4096):
    theta = W_t x + b_t            # queries  [C, HW]
    phi   = W_p x + b_p            # keys     [C, HW]
    g     = x                      # values   [C, HW]
    S     = theta^T phi / sqrt(C)  # [HW, HW]
    A     = softmax(S, axis=keys)
    f     = g A^T                  # [C, HW]
    out   = x + W_c f + b_c

Sharding: 8 cores; each of the N=4 batch elements is split across 2 cores
by query position (2048 queries per core). Every core holds the full key
set for its batch element, so no collectives are needed.

Per-core device program (SPMD, identical on all cores, data differs):
  - ALL matmuls run in fp8e4 DoubleRow mode (2 contraction k-tiles per
    instruction, 2 fp8 MACs/cell/cycle): the 256-channel contractions
    (projections, scores) are one DR matmul each; the 4096-key PV
    contraction is 16 DR matmuls per 512-query group.
  - scores are computed TRANSPOSED (S^T[k, q] = phi^T theta) so the exp
    output E^T feeds the PV matmul directly as the fp8 moving operand.
  - softmax uses exp(s - 3) (constant shift, softmax-invariant) so the
    max exp value ~96 stays under the TRN fp8e4 max of 240, and typical
    weights ~0.08 stay in the normal range.
  - the softmax denominator is a ones-column DR matmul per exp pair,
    accumulated in PSUM across the key loop (frees the Vector engine,
    which was the previous denominator path).
  - scale management for fp8: host prescales W by 16 (keeps W out of the
    fp8 subnormal range); evictions divide by 16 on DVE. f is scaled by
    16/sum (reciprocal folded with ln16 bias on ACT); the conv PSUM is
    divided by 256 in the final residual-add STT.
  - theta/phi evictions run on the Vector engine (tensor_scalar with
    1/16 scale + bias); exp and the reciprocal ln/exp run on ACT.
  - the attention loop is software-pipelined (PV trails scores/exp by one
    pair; per-group normalize/conv work is deferred into the next group).

Toolchain constraint that shapes this file: every TPB engine instruction
(and every DMA) may carry at most ONE semaphore wait, so cross-engine
fan-in is funneled through per-engine collector chains, persistent ring
tiles replace rotating tile pools, and loads/stores are merged so each
DMA is the first instruction on its hardware queue.

The host wrapper rolls x columns per-core so queries are always columns
[0, P) of the local key matrix (keeps the program identical across cores),
and pre-transposes x (and the weight matrices) since the PV matmul needs
x^T as the stationary operand.
"""

import math

import numpy as np

import concourse.bass as bass
import concourse.mybir as mybir
from concourse import bacc
from concourse.bass_utils import run_bass_kernel_spmd
from concourse.tile import TileContext, add_dep_helper


def _combined_ln_exp_tables(orig_fn):
    """Activation-table view that resolves Exp, Ln, Copy (and friends) to
    the single combined `natural_log_exp_and_others` set.

    The compiler's table-load pass picks the FIRST set containing each
    activation function; by default Exp resolves to `exp_and_others` and
    Ln to `natural_log`, so a kernel alternating exp and ln reloads the
    ACT table RAMs (~1.3us each) at every alternation. Hiding those
    functions from every other set makes all of this kernel's activations
    resolve to one set -> exactly one table load. Set ids keep their
    act_info.json positions, so the emitted act_func_set_id stays valid.
    """
    keep = "natural_log_exp_and_others"
    AFT = mybir.ActivationFunctionType
    hide = {AFT.Exp, AFT.Ln, AFT.Identity, AFT.Copy, AFT.MemsetZero}

    def patched(arch):
        tabs = orig_fn(arch)
        return {
            name: (funcs if name == keep else funcs - hide)
            for name, funcs in tabs.items()
        }

    return patched

N, C, H, W = 4, 256, 64, 64
HW = H * W
NCORES = 8
CORES_PER_N = NCORES // N
P_CORE = HW // CORES_PER_N  # queries per core

F32 = mybir.dt.float32
F16 = mybir.dt.float16
FP8 = mybir.dt.float8e4
DR = mybir.MatmulPerfMode.DoubleRow

EXP_SHIFT = -3.0  # softmax-invariant shift keeping exp under fp8e4 max 240
W_SCALE = 16.0    # host premultiplies weights (fp8 subnormal avoidance)
F_SCALE = 16.0    # f is stored as 16*f_normalized


def build_program(P, Q, Cc=C):
    """Build the per-core Bass program.

    P: queries handled by this core (first P columns of xk)
    Q: total key positions
    """
    assert P % 512 == 0 and Q % 512 == 0 and Cc == 256
    CT = Cc // 128  # = 2: one DoubleRow pair
    QT = Q // 128
    PG = P // 512
    QG = Q // 512
    scale = float(Cc) ** -0.5

    nc = bacc.Bacc("TRN2", target_bir_lowering=False)
    xk = nc.declare_dram_parameter("xk", [Cc, Q], FP8, isOutput=False)[:]
    xqb = nc.declare_dram_parameter("xqb", [Cc, P], F16, isOutput=False)[:]
    xt = nc.declare_dram_parameter("xt", [Q, Cc], FP8, isOutput=False)[:]
    wcat = nc.declare_dram_parameter("wcat", [3, Cc, Cc], FP8, isOutput=False)[:]
    bcat = nc.declare_dram_parameter("bcat", [3, Cc], F32, isOutput=False)[:]
    out = nc.declare_dram_parameter("out", [Cc, P], F16, isOutput=True)[:]

    add = mybir.AluOpType.add
    mult = mybir.AluOpType.mult

    with TileContext(nc) as tc:
        with (
            tc.tile_pool(name="const", bufs=1) as const,
            tc.tile_pool(name="big", bufs=1) as big,
            tc.tile_pool(name="pss", bufs=1, space="PSUM") as pss,
            tc.tile_pool(name="psf", bufs=1, space="PSUM") as psf,
            tc.tile_pool(name="pso", bufs=1, space="PSUM") as pso,
        ):
            QH = Q // 2

            # ---- input loads: 6 DMAs, one per HWDGE proc ----
            ws_sb = const.tile([128, 3, CT, Cc], FP8, tag="ws")
            w_load = nc.sync.dma_start(
                out=ws_sb, in_=wcat.rearrange("w (a p) o -> p w a o", p=128)
            )
            xk_sb = big.tile([128, CT, Q], FP8, tag="xk")
            xk_loads = [
                nc.sync.dma_start(
                    out=xk_sb[:, :, h * QH : (h + 1) * QH],
                    in_=xk[:, h * QH : (h + 1) * QH].rearrange(
                        "(a p) q -> p a q", p=128
                    ),
                )
                for h in range(2)
            ]
            bb = const.tile([128, 3, CT], F32, tag="bb")
            b_load = nc.sync.dma_start(
                out=bb, in_=bcat.rearrange("w (a p) -> p w a", p=128)
            )
            # xt/xqb are not needed until the attention loop; gating their
            # transfers on the xk halves gives xk full HBM bandwidth so the
            # first projection matmul can start ~5us earlier.
            xt_sb = big.tile([128, QT, Cc], FP8, tag="xt")
            xt_load = nc.sync.dma_start(
                out=xt_sb, in_=xt.rearrange("(a p) c -> p a c", p=128)
            )
            add_dep_helper(xt_load.ins, xk_loads[0].ins, True, "defer xt dma")
            xq_sb = big.tile([128, CT, P], F16, tag="xq")
            xq_load = nc.sync.dma_start(
                out=xq_sb, in_=xqb.rearrange("(a p) q -> p a q", p=128)
            )
            add_dep_helper(xq_load.ins, xk_loads[1].ins, True, "defer xqb dma")

            # persistent tiles (deliberately NOT pool-rotated: pool-slot
            # releases fan in multiple procs; rings keep wait fan-in low)
            th_sb = big.tile([128, CT, P], FP8, tag="th")
            ph_sb = big.tile([128, CT, Q], FP8, tag="ph")
            f_sb = big.tile([128, CT, P], FP8, tag="f")
            e_ring = big.tile([128, 4, 2, 512], FP8, tag="ering")
            rc_ring = const.tile([1, PG, 512], F16, tag="rcring")
            lns = const.tile([1, PG, 512], F32, tag="lns")
            bc_ring = big.tile([128, PG, 512], F32, tag="bcring")
            o_ring = big.tile([128, CT, PG, 512], F16, tag="oring")

            # ---- engine program-order chains + wait collectors ----
            last = {}

            def chain(eng, inst):
                # ordering edges disabled: Bacc legalizes multi-waits, so the
                # Tile scheduler is free to interleave within each engine
                last[eng] = inst.ins
                return inst

            ones_f = const.tile([128, 1], F32, tag="ones_f")
            chain("v", nc.vector.memset(ones_f, 1.0))
            ones_col = const.tile([1, 128], F16, tag="ones_col")
            chain("v", nc.vector.tensor_copy(
                ones_col, ones_f[0:1, 0:1].to_broadcast([1, 128])))
            ones8 = const.tile([128, 2, 16], FP8, tag="ones8")
            chain("v", nc.vector.memset(ones8, 1.0))
            zbias = const.tile([128, 1], F32, tag="zbias")
            zb_inst = chain("v", nc.vector.memset(zbias, 0.0))
            mbias = const.tile([128, 1], F32, tag="mbias")
            chain("v", nc.vector.memset(mbias, EXP_SHIFT))
            lbias = const.tile([128, 1], F32, tag="lbias")
            chain("v", nc.vector.memset(lbias, math.log(F_SCALE)))

            scr_act = const.tile([1, 1], F32, tag="scr_act")
            acol = nc.scalar.activation(
                scr_act, zbias[0:1, :], mybir.ActivationFunctionType.Copy
            )
            add_dep_helper(acol.ins, zb_inst.ins, True, "act bias barrier")
            last["a"] = acol.ins

            for k, ld in enumerate([b_load, xq_load]):
                scr_k = const.tile([1, 1], F32, tag=f"scr{k}", name=f"scr{k}")
                dcol = nc.vector.memset(scr_k, 0.0)
                add_dep_helper(dcol.ins, ld.ins, True, "dve input barrier")
                chain("v", dcol)

            ps_col = pso.tile([1, 1], F32, tag="den", name="ps_col")
            probe = bb[0:1, 0, 0:1]

            def pe_barrier(ld):
                col = nc.tensor.matmul(ps_col, lhsT=probe, rhs=probe)
                add_dep_helper(col.ins, ld.ins, True, "pe input barrier")
                chain("p", col)

            pe_barrier(w_load)
            pe_barrier(xk_loads[0])

            def mm(*args, **kwargs):
                return chain("p", nc.tensor.matmul(*args, **kwargs))

            def dve(fn, *args, **kwargs):
                return chain("v", fn(*args, **kwargs))

            def act(*args, **kwargs):
                return chain("a", nc.scalar.activation(*args, **kwargs))

            # ---- projections: one DR matmul per 512-col group; DVE
            # eviction applies the 1/16 weight-prescale compensation + bias
            def project(w_idx, dst, ngroups, bias_col, g0=0):
                for co in range(CT):
                    for g in range(g0, ngroups):
                        ps_pj = psf.tile(
                            [128, 512], F32, tag=f"f{g % 2}", name="ps_pj"
                        )
                        mm(
                            ps_pj,
                            lhsT=ws_sb[:, w_idx, :, co * 128 : (co + 1) * 128],
                            rhs=xk_sb[:, :, g * 512 : (g + 1) * 512],
                            perf_mode=DR,
                        )
                        dve(
                            nc.vector.tensor_scalar,
                            dst[:, co, g * 512 : (g + 1) * 512],
                            ps_pj,
                            1.0 / W_SCALE,
                            bb[:, bias_col, co : co + 1],
                            op0=mult,
                            op1=add,
                        )

            project(0, th_sb, PG, 0)
            project(1, ph_sb, QG // 2, 1)
            pe_barrier(xk_loads[1])
            project(1, ph_sb, QG, 1, g0=QG // 2)
            pe_barrier(xt_load)

            # ---- attention; per-group finalization is deferred into the
            # next group so the reciprocal chain (PSUM den -> ACT ln/exp ->
            # PE broadcast) overlaps PE work
            deferred = [None]

            def finalize_bc(pg):
                psl = slice(pg * 512, (pg + 1) * 512)
                ps_fs = deferred[0][1]
                ps_bc = pso.tile([128, 512], F32, tag="po", name="ps_bc")
                mm(ps_bc, lhsT=ones_col, rhs=rc_ring[:, pg, :])
                bc_sb = bc_ring[:, pg, :]
                dve(nc.vector.tensor_copy, bc_sb, ps_bc)
                for ci in range(CT):
                    dve(
                        nc.vector.tensor_mul, f_sb[:, ci, psl], ps_fs[ci], bc_sb
                    )

            def finalize_conv(pg):
                psl = slice(pg * 512, (pg + 1) * 512)
                for co in range(CT):
                    ps_o = pso.tile([128, 512], F32, tag="po", name="ps_o")
                    mm(
                        ps_o,
                        lhsT=ws_sb[:, 2, :, co * 128 : (co + 1) * 128],
                        rhs=f_sb[:, :, psl],
                        perf_mode=DR,
                    )
                    dve(
                        nc.vector.scalar_tensor_tensor,
                        out=o_ring[:, co, pg, :],
                        in0=ps_o,
                        scalar=1.0 / (W_SCALE * F_SCALE),
                        in1=xq_sb[:, co, psl],
                        op0=mult,
                        op1=add,
                    )
                deferred[0] = None

            for pg in range(PG):
                psl = slice(pg * 512, (pg + 1) * 512)
                ps_f = [
                    psf.tile([128, 512], F32, tag=f"f{ci}", name=f"ps_f{ci}")
                    for ci in range(CT)
                ]
                ps_den = pso.tile([1, 512], F32, tag="den", name="ps_den")
                # software pipeline: PV runs one exp-pair behind scores so
                # the PE streams scores(k+1) while ACT computes exp(k)
                for qp in range(QT // 2 + 1):
                    if qp < QT // 2:
                        ps_s = pss.tile([128, 2, 512], F32, tag="s", bufs=2)
                        for sub in range(2):
                            qt = qp * 2 + sub
                            mm(
                                ps_s[:, sub],
                                lhsT=ph_sb[:, :, qt * 128 : (qt + 1) * 128],
                                rhs=th_sb[:, :, psl],
                                perf_mode=DR,
                            )
                        act(
                            e_ring[:, qp % 4], ps_s,
                            mybir.ActivationFunctionType.Exp,
                            bias=mbias, scale=scale,
                        )
                    if qp == 1 and deferred[0] is not None:
                        finalize_bc(pg - 1)
                    if qp == 2 and deferred[0] is not None:
                        finalize_conv(pg - 1)
                    if qp >= 1:
                        qpp = qp - 1
                        e_p = e_ring[:, qpp % 4]
                        first, last_q = qpp == 0, qpp == QT // 2 - 1
                        mm(
                            ps_den,
                            lhsT=ones8[:, :, 0:1],
                            rhs=e_p,
                            perf_mode=DR,
                            start=first,
                            stop=last_q,
                        )
                        for ci in range(CT):
                            mm(
                                ps_f[ci],
                                lhsT=xt_sb[
                                    :, 2 * qpp : 2 * qpp + 2,
                                    ci * 128 : (ci + 1) * 128,
                                ],
                                rhs=e_p,
                                perf_mode=DR,
                                start=first,
                                stop=last_q,
                            )

                # reciprocal: 16/sum via ln+exp (one ACT table set)
                act(
                    lns[:, pg], ps_den, mybir.ActivationFunctionType.Ln,
                    bias=zbias[0:1],
                )
                act(
                    rc_ring[:, pg, :], lns[:, pg],
                    mybir.ActivationFunctionType.Exp,
                    bias=lbias[0:1], scale=-1.0,
                )
                deferred[0] = (pg, ps_f)

            finalize_bc(PG - 1)
            finalize_conv(PG - 1)

            # ---- output stores: one contiguous DMA per channel tile ----
            for co in range(CT):
                nc.sync.dma_start(
                    out=out[co * 128 : (co + 1) * 128, :], in_=o_ring[:, co]
                )

    orig_gat = bacc.get_activation_tables
    bacc.get_activation_tables = _combined_ln_exp_tables(orig_gat)
    try:
        nc.compile()
    finally:
        bacc.get_activation_tables = orig_gat
    return nc


_PROGRAM_CACHE = {}


def _get_program(mm_dt=None):
    key = "fp8dr"
    if key not in _PROGRAM_CACHE:
        _PROGRAM_CACHE[key] = build_program(P_CORE, HW, C)
    return _PROGRAM_CACHE[key]


def make_in_maps(x, theta_w, theta_b, phi_w, phi_b, conv1_w, conv1_b,
                 mm_np=None):
    """Host-side sharding / layout prep (pure data movement + prescale)."""
    fp8 = mybir.dt.np(FP8)
    wcat = np.ascontiguousarray(
        np.clip(
            W_SCALE * np.stack(
                [
                    np.asarray(theta_w, np.float32).T,
                    np.asarray(phi_w, np.float32).T,
                    np.asarray(conv1_w, np.float32).T,
                ]
            ),
            -240.0, 240.0,
        ).astype(fp8)
    )
    bcat = np.ascontiguousarray(
        np.stack(
            [
                np.asarray(theta_b, np.float32),
                np.asarray(phi_b, np.float32),
                np.asarray(conv1_b, np.float32),
            ]
        )
    )
    xf = np.asarray(x, np.float32).reshape(N, C, HW)
    cb = np.asarray(conv1_b, np.float32)[:, None]
    in_maps = []
    for core in range(NCORES):
        n, half = divmod(core, CORES_PER_N)
        off = half * P_CORE
        xk_i = np.ascontiguousarray(np.roll(xf[n], -off, axis=1))
        in_maps.append(
            {
                "xk": xk_i.astype(fp8),
                "xqb": (xk_i[:, :P_CORE] + cb).astype(np.float16),
                "xt": np.ascontiguousarray(xk_i.T).astype(fp8),
                "wcat": wcat,
                "bcat": bcat,
            }
        )
    return in_maps


def assemble_output(results):
    y = np.empty((N, C, HW), np.float32)
    for core in range(NCORES):
        n, half = divmod(core, CORES_PER_N)
        off = half * P_CORE
        y[n][:, off : off + P_CORE] = results[core]["out"].astype(np.float32)
    return y.reshape(N, C, H, W)


def kernel(x, theta_w, theta_b, phi_w, phi_b, conv1_w, conv1_b,
           mm_dt=None, **run_kwargs):
    nc = _get_program()
    in_maps = make_in_maps(
        x, theta_w, theta_b, phi_w, phi_b, conv1_w, conv1_b
    )
    res = run_bass_kernel_spmd(nc, in_maps, list(range(NCORES)), **run_kwargs)
    out = assemble_output(res.results)
    kernel.last_results = res
    return out
